# revision 1
# baseline (speedup 1.0000x reference)
"""Trainium2 Bass kernel for nn_Loss_20993800143146 (loss_fn).

Computes, over 8 NeuronCores (data-parallel over batch / bh):
    mel_loss  = mean(|mels_pred * mask - mels_target|)           (mean over full tensor)
    stop_loss = sum(-5 * clamp(log(stop_pred[b, last_idx_b]), -100)) / mask.sum()
    dc        = sum(alignments * band[s,t] * bmask[b]) / (H * lengths.sum() * N)
    out       = mel_loss + stop_loss - 1e-4 * dc

Key algebraic fact: band[s,t] = (s >= clip(5t-50,0,160)) & (s < clip(5t+50,0,160))
is identically zero for t >= 42 (clip hits s=160), so only alignments[:,:,:,:42]
is ever read (~5 MB of the 98 MB tensor).

Sharding: batch dim (16 -> 2 per core) for lengths/mask/stop/mels, bh dim
(64 -> 8 per core) for alignments. Each core reduces its shard to 8 partial
scalars on-device; the host sums the 8 partial vectors and applies the final
constant-denominator arithmetic.

Per-core layout: everything except the band weights lives in ONE f32 DRAM
tensor `bigf` [128, 3503] (columns, in f32 units):
    0:13     stop13S   stop_pred split per b: b0 -> partitions 0..63, b1 ->
                       64..127, 13 t's per partition (pad = 1.0 so Ln finite)
    13:26    iota13S   t+1 in that layout (0 = pad)
    26:154   ident     128x128 identity for PE transposes
    154:161  masks2    28 raw bytes: [0:13] mask in mel layout, [13:26] in
                       stop layout (bitcast u8 view)
    161:163  lens      2 int32: col0 lengths[b_local(p)], col1 lengths (p<16)
    163:1203 melst     mels_target rows (b,t) padded 1600->1664, 13 rows of
                       80 per partition
    1203:2243 melsp    mels_pred, same layout
    2243:3503 align    alignments shard [b_local, n, s, t<42], 16 partitions
                       per b_local, 30 rows of 42 per partition
`wband` [128,1260] u8 holds the band weight per align element (ACT-cast to
f32 on device). SP issues chunk1/melst/melsp, ACT issues wband/align halves
(separate 16-queue HWDGE sets, so issue + transfer run in parallel).

Stats tile [128,8] is reduced across partitions with one PE matmul vs ones:
  cols: 0=dc_w, 1=melA(sum m|d|), 2=melB(sum|b|), 3=melC(sum m|b|),
        4=mask_cnt, 5=logp_b0, 6=lengths_sum, 7=logp_b1.
"""

import numpy as np

# Problem constants (hardcoded per contract; kernel.py must be self-contained).
H = 4
B = 16
T = 800
NMEL = 80
S = 160
N = 3
BW = 50
K = T // S  # 5
TC = 42  # band[:, t] == 0 for all t >= TC
NCORES = 8

MEL_ROWS = 2 * T            # 1600 (b,t) rows per core
MEL_PAD_ROWS = 1664         # pad to 128 * 13
MG = 13                     # 80-col groups per partition (mel) / t's (stop)
ALN_F = N * S * TC // 16    # 1260 free elems per partition (8 b * 16 part/b)

# bigf column layout (f32 units)
C_STOP = 0
C_IOTA = MG
C_ID = 2 * MG            # 26
C_MK = C_ID + 128        # 154 (7 f32 = 28 bytes, 26 used)
C_LEN = C_MK + 7         # 161 (2 i32)
C_MT = C_LEN + 2         # 163
C_MP = C_MT + MG * NMEL  # 1203
C_AL = C_MP + MG * NMEL  # 2243
BIGF = C_AL + ALN_F      # 3503
AL_HALF = ALN_F // 2     # 630

_CACHE = {}


def _band():
    tr = np.arange(TC)
    mn = np.clip(K * tr - BW, 0, S)
    mx = np.clip(K * tr + BW, 0, S)
    rows = np.arange(S)
    return ((rows[:, None] >= mn[None, :]) & (rows[:, None] < mx[None, :]))


def _wband_u8():
    """Band weight tile [128, 1260]: partition p holds rows (p%16)*30+j of the
    (n, s) x t[:TC] block of one b; weight depends only on s = row % 160."""
    band = _band()  # [S, TC] bool
    p_idx = np.arange(128)
    j_idx = np.arange(30)
    s_of = (((p_idx[:, None] % 16) * 30) + j_idx[None, :]) % S  # [128, 30]
    return band[s_of].reshape(128, ALN_F).astype(np.uint8)


def _iota13s():
    """[128,13] f32: t+1 in the stop split layout, 0 in pad positions."""
    out = np.zeros((128, MG), np.float32)
    for p in range(128):
        base = 13 * (p % 64)
        for j in range(MG):
            t = base + j
            if t < T:
                out[p, j] = t + 1
    return out


def _split13(row, pad_value):
    """[800] -> [64,13] padded with pad_value."""
    out = np.full((64 * MG,), pad_value, row.dtype)
    out[:T] = row
    return out.reshape(64, MG)


def _build_bass():
    import concourse.bacc as bacc
    import concourse.tile as tile
    import concourse.mybir as mybir
    from contextlib import ExitStack

    f32 = mybir.dt.float32
    u8 = mybir.dt.uint8
    i32 = mybir.dt.int32
    Alu = mybir.AluOpType
    Act = mybir.ActivationFunctionType
    Ax = mybir.AxisListType

    nc = bacc.Bacc("TRN2", target_bir_lowering=False, debug=False,
                   num_devices=NCORES)

    bigf = nc.dram_tensor("bigf", [128, BIGF], f32, kind="ExternalInput").ap()
    wband = nc.dram_tensor("wband", [128, ALN_F], u8, kind="ExternalInput").ap()
    out = nc.dram_tensor("out", [8, 1], f32, kind="ExternalOutput").ap()

    with tile.TileContext(nc) as tc:
        with ExitStack() as ctx:
            pool = ctx.enter_context(tc.tile_pool(name="main", bufs=1))
            ppool = ctx.enter_context(tc.tile_pool(name="ps", bufs=1, space="PSUM"))

            big_t = pool.tile([128, BIGF], f32, tag="big")
            wb_t = pool.tile([128, ALN_F], u8, tag="wb")
            wf_t = pool.tile([128, ALN_F], f32, tag="wf")

            # ---- DMA issues: SP and ACT have separate HWDGE queue sets ----
            nc.sync.dma_start(big_t[:, 0:C_MT], bigf[:, 0:C_MT])
            nc.scalar.dma_start(wb_t[:], wband)
            nc.sync.dma_start(big_t[:, C_MT:C_MP], bigf[:, C_MT:C_MP])
            nc.sync.dma_start(big_t[:, C_MP:C_AL], bigf[:, C_MP:C_AL])
            nc.scalar.dma_start(big_t[:, C_AL:C_AL + AL_HALF],
                                bigf[:, C_AL:C_AL + AL_HALF])
            nc.scalar.dma_start(big_t[:, C_AL + AL_HALF:BIGF],
                                bigf[:, C_AL + AL_HALF:BIGF])

            # stats[:, c]: 0=dc_w, 1=melA, 2=melB, 3=melC, 4=mask_cnt,
            # 5=logp_b0, 6=len_sum, 7=logp_b1
            st_t = pool.tile([128, 8], f32, tag="st")
            nc.vector.memset(st_t[:], 0.0)
            on_t = pool.tile([128, 1], f32, tag="on")
            nc.vector.memset(on_t[:], 1.0)

            stop_v = big_t[:, C_STOP:C_STOP + MG]
            iota_v = big_t[:, C_IOTA:C_IOTA + MG]
            id_v = big_t[:, C_ID:C_ID + 128]
            mk_v = big_t[:, C_MK:C_MK + 7].bitcast(u8)     # [128, 28]
            len_v = big_t[:, C_LEN:C_LEN + 2].bitcast(i32)  # [128, 2]
            mt_v = big_t[:, C_MT:C_MP].rearrange("p (g m) -> p g m", m=NMEL)
            mp_v = big_t[:, C_MP:C_AL].rearrange("p (g m) -> p g m", m=NMEL)
            al_v = big_t[:, C_AL:BIGF]

            # band-weight u8 -> f32 cast on the scalar engine
            nc.scalar.activation(wf_t[:], wb_t[:], Act.Copy)

            # ---- stop term stage A (b0 on partitions 0:64, b1 on 64:128) ----
            lp_t = pool.tile([128, MG], f32, tag="lp")
            nc.scalar.activation(lp_t[:], stop_v, Act.Ln)
            cl_t = pool.tile([128, MG], f32, tag="cl")
            nc.vector.tensor_scalar_max(cl_t[:], lp_t[:], -100.0)
            msf_t = pool.tile([128, MG], f32, tag="msf")
            nc.vector.tensor_copy(msf_t[:], mk_v[:, MG:2 * MG])
            m13f_t = pool.tile([128, MG], f32, tag="m13f")
            nc.vector.tensor_copy(m13f_t[:], mk_v[:, 0:MG])
            tl_t = pool.tile([128, MG], f32, tag="tl")
            nc.vector.tensor_mul(tl_t[:], iota_v, msf_t[:])
            mxp_t = pool.tile([128, 1], f32, tag="mxp")
            nc.vector.tensor_reduce(mxp_t[:], tl_t[:], axis=Ax.X, op=Alu.max)
            eqj_t = pool.tile([128, MG], f32, tag="eqj")
            cp_t = pool.tile([128, 1], f32, tag="cp")
            nc.vector.scalar_tensor_tensor(
                eqj_t[:], tl_t[:], mxp_t[:, 0:1], cl_t[:],
                op0=Alu.is_equal, op1=Alu.mult, accum_out=cp_t[:])
            nc.vector.tensor_reduce(st_t[:, 4:5], m13f_t[:], axis=Ax.X, op=Alu.add)

            # ---- lengths (tiny, data arrives with chunk 1) ----
            lrf_t = pool.tile([128, 1], f32, tag="lrf")
            nc.vector.tensor_copy(lrf_t[:], len_v[:, 0:1])
            nc.vector.tensor_copy(st_t[:, 6:7], len_v[:, 1:2])
            bm_t = pool.tile([128, 1], f32, tag="bm")
            nc.vector.tensor_scalar(bm_t[:], lrf_t[:], float(T), None, op0=Alu.is_le)

            # ---- mel term ----
            v2_t = pool.tile([128, MG], f32, tag="v2")
            nc.vector.tensor_reduce(v2_t[:], mt_v, axis=Ax.X, op=Alu.add,
                                    apply_absolute_value=True)
            d_t = pool.tile([128, MG * NMEL], f32, tag="d")
            nc.vector.tensor_sub(d_t[:], mp_v, mt_v)
            v1_t = pool.tile([128, MG], f32, tag="v1")
            nc.vector.tensor_reduce(
                v1_t[:], d_t[:].rearrange("p (g m) -> p g m", m=NMEL),
                axis=Ax.X, op=Alu.add, apply_absolute_value=True)
            w1_t = pool.tile([128, MG], f32, tag="w1")
            nc.vector.scalar_tensor_tensor(
                w1_t[:], v1_t[:], 1.0, m13f_t[:],
                op0=Alu.bypass, op1=Alu.mult, accum_out=st_t[:, 1:2])
            nc.vector.tensor_reduce(st_t[:, 2:3], v2_t[:], axis=Ax.X, op=Alu.add)
            w2_t = pool.tile([128, MG], f32, tag="w2")
            nc.vector.scalar_tensor_tensor(
                w2_t[:], v2_t[:], 1.0, m13f_t[:],
                op0=Alu.bypass, op1=Alu.mult, accum_out=st_t[:, 3:4])

            # ---- dc term (two halves so compute overlaps the 2nd DMA) ----
            pra_t = pool.tile([128, AL_HALF], f32, tag="pra")
            dca_t = pool.tile([128, 1], f32, tag="dca")
            nc.vector.scalar_tensor_tensor(
                pra_t[:], al_v[:, 0:AL_HALF], 1.0, wf_t[:, 0:AL_HALF],
                op0=Alu.bypass, op1=Alu.mult, accum_out=dca_t[:])
            prb_t = pool.tile([128, AL_HALF], f32, tag="prb")
            dcb_t = pool.tile([128, 1], f32, tag="dcb")
            nc.vector.scalar_tensor_tensor(
                prb_t[:], al_v[:, AL_HALF:ALN_F], 1.0, wf_t[:, AL_HALF:ALN_F],
                op0=Alu.bypass, op1=Alu.mult, accum_out=dcb_t[:])
            dcs_t = pool.tile([128, 1], f32, tag="dcs")
            nc.vector.tensor_add(dcs_t[:], dca_t[:], dcb_t[:])
            nc.vector.tensor_mul(st_t[:, 0:1], dcs_t[:], bm_t[:])

            # ---- stop stage B: transpose Mp and cp into the free dim on PE,
            # then per-b max + select on partition 0 only.
            psA = ppool.tile([1, 128], f32, tag="psA")
            nc.tensor.transpose(psA[:], mxp_t[:], id_v)
            psB = ppool.tile([1, 128], f32, tag="psB")
            nc.tensor.transpose(psB[:], cp_t[:], id_v)
            sbA_t = pool.tile([1, 128], f32, tag="sbA")
            nc.vector.tensor_copy(sbA_t[:], psA[:])
            mb0_t = pool.tile([1, 1], f32, tag="mb0")
            nc.vector.tensor_reduce(mb0_t[:], sbA_t[0:1, 0:64], axis=Ax.X, op=Alu.max)
            mb1_t = pool.tile([1, 1], f32, tag="mb1")
            nc.vector.tensor_reduce(mb1_t[:], sbA_t[0:1, 64:128], axis=Ax.X, op=Alu.max)
            ej0_t = pool.tile([1, 64], f32, tag="ej0")
            nc.vector.scalar_tensor_tensor(
                ej0_t[:], sbA_t[0:1, 0:64], mb0_t[:, 0:1], psB[0:1, 0:64],
                op0=Alu.is_equal, op1=Alu.mult, accum_out=st_t[0:1, 5:6])
            ej1_t = pool.tile([1, 64], f32, tag="ej1")
            nc.vector.scalar_tensor_tensor(
                ej1_t[:], sbA_t[0:1, 64:128], mb1_t[:, 0:1], psB[0:1, 64:128],
                op0=Alu.is_equal, op1=Alu.mult, accum_out=st_t[0:1, 7:8])

            # ---- partition reduction via PE: out[8,1] = stats.T @ ones ----
            pt = ppool.tile([8, 1], f32, tag="pt")
            nc.tensor.matmul(pt[:], lhsT=st_t[:], rhs=on_t[:],
                             start=True, stop=True)
            ex_t = pool.tile([8, 1], f32, tag="ex")
            nc.vector.tensor_copy(ex_t[:], pt[:])
            nc.sync.dma_start(out, ex_t[:])

    nc.compile()
    return nc


def _get_nc():
    if "nc" not in _CACHE:
        _CACHE["nc"] = _build_bass()
    return _CACHE["nc"]


def make_in_maps(lengths, mask, stop_pred, mels_pred, mels_target, alignments):
    """Shard full inputs into the 8 per-core input dicts."""
    lengths = np.ascontiguousarray(lengths, dtype=np.int32)
    mask_u8 = np.ascontiguousarray(mask).view(np.uint8) if mask.dtype == np.bool_ \
        else np.ascontiguousarray(mask.astype(np.uint8))
    stop_pred = np.ascontiguousarray(stop_pred, dtype=np.float32)
    mels_pred = np.ascontiguousarray(mels_pred, dtype=np.float32)
    mels_target = np.ascontiguousarray(mels_target, dtype=np.float32)
    alignments = np.ascontiguousarray(alignments, dtype=np.float32)

    wband = _wband_u8()
    iota13s = _iota13s()
    ident = np.eye(128, dtype=np.float32)

    def pad_rows(x2d, cols):
        padded = np.zeros((MEL_PAD_ROWS, cols), x2d.dtype)
        padded[:MEL_ROWS] = x2d
        return padded

    in_maps = []
    for c in range(NCORES):
        bs = slice(2 * c, 2 * c + 2)
        bigf = np.zeros((128, BIGF), np.float32)
        bigf[:, C_STOP:C_STOP + MG] = np.concatenate(
            [_split13(stop_pred[2 * c], np.float32(1.0)),
             _split13(stop_pred[2 * c + 1], np.float32(1.0))])
        bigf[:, C_IOTA:C_IOTA + MG] = iota13s
        bigf[:, C_ID:C_ID + 128] = ident
        mk_bytes = bigf[:, C_MK:C_MK + 7].view(np.uint8).reshape(128, 28)
        mk_bytes[:, 0:MG] = pad_rows(mask_u8[bs].reshape(MEL_ROWS, 1), 1).reshape(128, MG)
        mk_bytes[:, MG:2 * MG] = np.concatenate(
            [_split13(mask_u8[2 * c], np.uint8(0)),
             _split13(mask_u8[2 * c + 1], np.uint8(0))])
        b_lo = 8 * (c % 2)
        len_i32 = bigf[:, C_LEN:C_LEN + 2].view(np.int32).reshape(128, 2)
        len_i32[:, 0] = np.repeat(lengths[b_lo:b_lo + 8], 16)
        len_i32[:B, 1] = lengths
        bigf[:, C_MT:C_MP] = \
            pad_rows(mels_target[bs].reshape(MEL_ROWS, NMEL), NMEL).reshape(128, MG * NMEL)
        bigf[:, C_MP:C_AL] = \
            pad_rows(mels_pred[bs].reshape(MEL_ROWS, NMEL), NMEL).reshape(128, MG * NMEL)
        bigf[:, C_AL:BIGF] = np.ascontiguousarray(
            alignments[:, 8 * c:8 * c + 8, :, :TC].transpose(1, 0, 2, 3)
        ).reshape(128, ALN_F)

        in_maps.append({"bigf": bigf, "wband": wband})
    return in_maps


def combine_partials(partials):
    """partials: list of 8 arrays [8,1] -> final scalar (0-d f32 ndarray)."""
    ps = np.stack([np.asarray(p, dtype=np.float64).reshape(8) for p in partials])
    dc_w = ps[:, 0].sum()
    mel_num = ps[:, 1].sum() + ps[:, 2].sum() - ps[:, 3].sum()
    logp = ps[:, 5].sum() + ps[:, 7].sum()
    mask_cnt = ps[:, 4].sum()
    len_sum = ps[0, 6]
    mel_loss = mel_num / float(B * T * NMEL)
    stop_loss = -5.0 * logp / mask_cnt
    dc = dc_w / (H * len_sum * N)
    return np.array(np.float32(mel_loss + stop_loss - 1e-4 * dc))


def kernel(lengths, mask, stop_pred, mels_pred, mels_target, alignments):
    from concourse.bass_utils import run_bass_kernel_spmd

    nc = _get_nc()
    in_maps = make_in_maps(lengths, np.asarray(mask), stop_pred,
                           mels_pred, mels_target, alignments)
    res = run_bass_kernel_spmd(nc, in_maps, list(range(NCORES)))
    return combine_partials([r["out"] for r in res.results])



# revision 5
# speedup vs baseline: 1.0809x; 1.0809x over previous
"""Trainium2 Bass kernel for nn_Loss_20993800143146 (loss_fn).

Computes, over 8 NeuronCores (data-parallel over batch / bh):
    mel_loss  = mean(|mels_pred * mask - mels_target|)           (mean over full tensor)
    stop_loss = sum(-5 * log(stop_pred[b, last_idx_b])) / mask.sum()
    dc        = sum(alignments * band[s,t] * bmask[b]) / (H * lengths.sum() * N)
    out       = mel_loss + stop_loss - 1e-4 * dc

Key algebraic facts:
  * band[s,t] = (s >= clip(5t-50,0,160)) & (s < clip(5t+50,0,160)) is zero for
    t >= 42, and within t < 42 only 2975 of the 6720 (s,t) cells are nonzero.
    The host packs EXACTLY the banded elements densely (zero-padded to a
    rectangle), so the device just sums them - no band weights needed.
  * The mel mask multiplies mels_pred only, so it is folded into the host-side
    packing of the pred tile (masked positions packed as 0), leaving a plain
    sum(|p - t|) on device.

Sharding: batch dim (16 -> 2 per core) for mask/stop/mels, bh dim (64 -> 8 per
core) for alignments. Each core reduces its shard to 8 partial f32 stats; the
host sums the 8x8 partials and applies the constant-denominator arithmetic.

Per-core inputs (heavy data in bf16/f16 - rel-err budget is 2e-2, measured
error stays ~1e-4):
  dA bf16 [128, 1320]: cols 0:1040 mels_pred*mask rows (b,t) padded 1600->1664,
                       13 rows of 80 per partition; cols 1040:1320 align half A.
  dB u8  [128, 2688]: bytes 0:2080 mels_target bf16 (same row layout);
                      2080:2640 align half B bf16; 2640:2644 f32 length of this
                      partition's b; 2656:2688 stats prefill (8 f32: all zero
                      except col 4 = lengths at partitions 0/1).
  dS f16 [2, 2400]:   per-b rows: 0:800 stop_pred, 800:1600 mask, 1600:2400
                      iota (t+1).
Alignments: partition p = 16*b_local + i holds banded elements of b's 3 heads,
flattened (n,s,t)-major, split 560 -> 280+280 between dA and dB.

Stats cols: 0=dc_w, 1=melA (sum|p*m-t|), 2=sel_lnp, 3=mask_cnt, 4=len, 5-7=0.
A GpSimd cross-partition reduce collapses [128,8] -> [1,8], DMA'd out.
"""

import numpy as np
import ml_dtypes

# Problem constants (hardcoded per contract; kernel.py must be self-contained).
H = 4
B = 16
T = 800
NMEL = 80
S = 160
N = 3
BW = 50
K = T // S  # 5
TC = 42  # band[:, t] == 0 for all t >= TC
NCORES = 8

MEL_ROWS = 2 * T            # 1600 (b,t) rows per core
MEL_PAD_ROWS = 1664         # pad to 128 * 13
MG = 13                     # 80-col groups per partition
MEL_F = MG * NMEL           # 1040 mel elements per partition per tensor
ALN_PER_PLANE = 2975        # nonzero band cells per (n, bh) plane
ALN_PER_PART = 560          # ceil(3*2975/16) padded: 16*560 >= 8925
ALN_HALF = ALN_PER_PART // 2  # 280

WA = MEL_F + ALN_HALF       # 1320 bf16 els per partition in dA
# dB byte offsets
BO_MT = 0
BO_ALN = 2 * MEL_F          # 2080
BO_LEN = BO_ALN + 2 * ALN_HALF  # 2640
BO_ST = 2656                # stats prefill, 16B aligned
WB = BO_ST + 32             # 2688 bytes

_CACHE = {}


def _band_bool():
    tr = np.arange(TC)
    mn = np.clip(K * tr - BW, 0, S)
    mx = np.clip(K * tr + BW, 0, S)
    rows = np.arange(S)
    return (rows[:, None] >= mn[None, :]) & (rows[:, None] < mx[None, :])


def _build_bass():
    import concourse.bacc as bacc
    import concourse.tile as tile
    import concourse.mybir as mybir
    from contextlib import ExitStack

    f32 = mybir.dt.float32
    f16 = mybir.dt.float16
    bf16 = mybir.dt.bfloat16
    u8 = mybir.dt.uint8
    Alu = mybir.AluOpType
    Act = mybir.ActivationFunctionType
    Ax = mybir.AxisListType

    nc = bacc.Bacc("TRN2", target_bir_lowering=False, debug=False,
                   num_devices=NCORES)

    dA = nc.dram_tensor("dA", [128, WA], bf16, kind="ExternalInput").ap()
    dB = nc.dram_tensor("dB", [128, WB], u8, kind="ExternalInput").ap()
    dS = nc.dram_tensor("dS", [2, 2400], f16, kind="ExternalInput").ap()
    out = nc.dram_tensor("out", [1, 8], f32, kind="ExternalOutput").ap()

    with tile.TileContext(nc) as tc:
        with ExitStack() as ctx:
            pool = ctx.enter_context(tc.tile_pool(name="main", bufs=1))

            ts_t = pool.tile([2, 2400], f16, tag="ts")
            ta_t = pool.tile([128, WA], bf16, tag="ta")
            tb_t = pool.tile([128, WB], u8, tag="tb")

            # DMA issues: sync (SP) and scalar (ACT) drive separate HWDGE
            # queue octets, so the two big tiles stream in parallel.
            nc.sync.dma_start(ts_t[:], dS)
            nc.sync.dma_start(ta_t[:], dA)
            nc.scalar.dma_start(tb_t[:], dB)

            stop_v = ts_t[:, 0:800]
            mask_v = ts_t[:, 800:1600]
            iota_v = ts_t[:, 1600:2400]
            mp_v = ta_t[:, 0:MEL_F]
            alnA_v = ta_t[:, MEL_F:WA]
            mt_v = tb_t[:, BO_MT:BO_ALN].bitcast(bf16)
            alnB_v = tb_t[:, BO_ALN:BO_LEN].bitcast(bf16)
            lenf_v = tb_t[:, BO_LEN:BO_LEN + 4].bitcast(f32)
            stats = tb_t[:, BO_ST:BO_ST + 32].bitcast(f32)   # [128, 8]

            # ---- stop term: per-b rows, no cross-partition traffic ----
            lnp_t = pool.tile([2, 800], f16, tag="lnp")
            nc.scalar.activation(lnp_t[:], stop_v, Act.Ln)
            tl_t = pool.tile([2, 800], f16, tag="tl")
            nc.vector.scalar_tensor_tensor(
                tl_t[:], iota_v, 1.0, mask_v, op0=Alu.bypass, op1=Alu.mult)
            mx_t = pool.tile([2, 1], f16, tag="mx")
            nc.vector.tensor_reduce(mx_t[:], tl_t[:], axis=Ax.X, op=Alu.max)
            eq_t = pool.tile([2, 800], f16, tag="eq")
            nc.vector.scalar_tensor_tensor(
                eq_t[:], tl_t[:], mx_t[:, 0:1], lnp_t[:],
                op0=Alu.is_equal, op1=Alu.mult, accum_out=stats[0:2, 2:3])
            nc.vector.tensor_reduce(stats[0:2, 3:4], mask_v, axis=Ax.X,
                                    op=Alu.add)

            # ---- mel term: d = p*m - t, then fused |.| + row-sum ----
            d_t = pool.tile([128, MEL_F], bf16, tag="d")
            nc.vector.tensor_sub(d_t[:], mp_v, mt_v)
            nc.vector.tensor_reduce(stats[:, 1:2], d_t[:], axis=Ax.X,
                                    op=Alu.add, apply_absolute_value=True)

            # ---- dc term: banded elements are pre-packed, just sum ----
            asum_t = pool.tile([128, ALN_HALF], bf16, tag="asum")
            dcs_t = pool.tile([128, 1], f32, tag="dcs")
            nc.vector.scalar_tensor_tensor(
                asum_t[:], alnA_v, 1.0, alnB_v,
                op0=Alu.bypass, op1=Alu.add, accum_out=dcs_t[:])
            bm_t = pool.tile([128, 1], f32, tag="bm")
            nc.vector.tensor_scalar(bm_t[:], lenf_v, float(T), None,
                                    op0=Alu.is_le)
            nc.vector.tensor_mul(stats[:, 0:1], dcs_t[:], bm_t[:])

            # ---- cross-partition reduce of the stats block, then out ----
            from concourse import bass_isa
            red_t = pool.tile([128, 8], f32, tag="red")
            nc.gpsimd.partition_all_reduce(red_t[:], stats, channels=128,
                                           reduce_op=bass_isa.ReduceOp.add)
            nc.sync.dma_start(out, red_t[0:1, :])

    nc.compile()
    return nc


def _get_nc():
    if "nc" not in _CACHE:
        _CACHE["nc"] = _build_bass()
    return _CACHE["nc"]


def make_in_maps(lengths, mask, stop_pred, mels_pred, mels_target, alignments):
    """Shard full inputs into the 8 per-core input dicts."""
    lengths = np.ascontiguousarray(lengths, dtype=np.int32)
    maskf = np.ascontiguousarray(mask).astype(np.float32)
    stop_pred = np.ascontiguousarray(stop_pred, dtype=np.float32)
    mels_pred = np.ascontiguousarray(mels_pred, dtype=np.float32)
    mels_target = np.ascontiguousarray(mels_target, dtype=np.float32)
    alignments = np.ascontiguousarray(alignments, dtype=np.float32)

    bf = ml_dtypes.bfloat16
    band = _band_bool()  # [S, TC]
    # Banded elements for every (n, bh) plane: [N, B*H, 2975]
    el = alignments[:, :, :, :TC][:, :, band]

    iota_f16 = (np.arange(T, dtype=np.float32) + 1.0).astype(np.float16)

    def pad_rows(x2d):
        padded = np.zeros((MEL_PAD_ROWS, NMEL), x2d.dtype)
        padded[:MEL_ROWS] = x2d
        return padded.reshape(128, MEL_F)

    in_maps = []
    for c in range(NCORES):
        bs = slice(2 * c, 2 * c + 2)
        # masked pred / raw target in (b,t)-row layout, bf16
        mp = (mels_pred[bs] * maskf[bs][..., None]).reshape(MEL_ROWS, NMEL)
        mt = mels_target[bs].reshape(MEL_ROWS, NMEL)
        mp_b = pad_rows(mp.astype(bf))
        mt_b = pad_rows(mt.astype(bf))

        # banded alignments: 8 local b's, 3 heads each -> [128, 560] bf16
        aln = np.zeros((8, 16 * ALN_PER_PART), bf)
        core_el = el[:, 8 * c:8 * c + 8]          # [3, 8, 2975]
        aln[:, :N * ALN_PER_PLANE] = \
            core_el.transpose(1, 0, 2).reshape(8, N * ALN_PER_PLANE).astype(bf)
        aln = aln.reshape(128, ALN_PER_PART)

        dA = np.empty((128, WA), bf)
        dA[:, :MEL_F] = mp_b
        dA[:, MEL_F:] = aln[:, :ALN_HALF]

        dB = np.zeros((128, WB), np.uint8)
        dB[:, BO_MT:BO_ALN] = mt_b.view(np.uint8)
        dB[:, BO_ALN:BO_LEN] = np.ascontiguousarray(
            aln[:, ALN_HALF:]).view(np.uint8)
        lenf = np.repeat(lengths[bs].astype(np.float32), 64)  # [128]
        dB[:, BO_LEN:BO_LEN + 4] = lenf[:, None].view(np.uint8)
        st = np.zeros((128, 8), np.float32)
        st[0:2, 4] = lengths[bs]
        dB[:, BO_ST:BO_ST + 32] = st.view(np.uint8)

        dS = np.zeros((2, 2400), np.float16)
        dS[:, 0:800] = stop_pred[bs].astype(np.float16)
        dS[:, 800:1600] = maskf[bs].astype(np.float16)
        dS[:, 1600:2400] = iota_f16[None, :]

        in_maps.append({"dA": dA, "dB": dB, "dS": dS})
    return in_maps


def combine_partials(partials):
    """partials: list of 8 arrays [1,8] -> final scalar (0-d f32 ndarray)."""
    ps = np.stack([np.asarray(p, dtype=np.float64).reshape(8)
                   for p in partials])
    tot = ps.sum(axis=0)
    dc_w, melA, sel_lnp, mask_cnt, len_sum = tot[0], tot[1], tot[2], tot[3], tot[4]
    mel_loss = melA / float(B * T * NMEL)
    stop_loss = -5.0 * sel_lnp / mask_cnt
    dc = dc_w / (H * len_sum * N)
    return np.array(np.float32(mel_loss + stop_loss - 1e-4 * dc))


def kernel(lengths, mask, stop_pred, mels_pred, mels_target, alignments):
    from concourse.bass_utils import run_bass_kernel_spmd

    nc = _get_nc()
    in_maps = make_in_maps(lengths, np.asarray(mask), stop_pred,
                           mels_pred, mels_target, alignments)
    res = run_bass_kernel_spmd(nc, in_maps, list(range(NCORES)))
    return combine_partials([r["out"] for r in res.results])


# revision 6
# speedup vs baseline: 1.1588x; 1.0721x over previous
"""Trainium2 Bass kernel for nn_Loss_20993800143146 (loss_fn).

Computes, over 8 NeuronCores (data-parallel over batch / bh):
    mel_loss  = mean(|mels_pred * mask - mels_target|)           (mean over full tensor)
    stop_loss = sum(-5 * log(stop_pred[b, last_idx_b])) / mask.sum()
    dc        = sum(alignments * band[s,t] * bmask[b]) / (H * lengths.sum() * N)
    out       = mel_loss + stop_loss - 1e-4 * dc

Key algebraic facts:
  * band[s,t] = (s >= clip(5t-50,0,160)) & (s < clip(5t+50,0,160)) is zero for
    t >= 42, and within t < 42 only 2975 of the 6720 (s,t) cells are nonzero.
    The host packs EXACTLY the banded elements densely (zero-padded to a
    rectangle), so the device just sums them - no band weights needed.
  * The mel mask multiplies mels_pred only, so it is folded into the host-side
    packing of the pred tile (masked positions packed as 0), leaving a plain
    sum(|p - t|) on device.

Sharding: batch dim (16 -> 2 per core) for mask/stop/mels, bh dim (64 -> 8 per
core) for alignments. Each core reduces its shard to 8 partial f32 stats; the
host sums the 8x8 partials and applies the constant-denominator arithmetic.

Heavy data in bf16 (mels) / fp8-e4m3 (alignments); rel-err budget is 2e-2,
measured error stays ~1e-4.

Per-core inputs, two DRAM tensors so the SP- and ACT-issued HWDGE queue octets
stream in parallel:
  dA u8 [128, 2360]: 0:2080 mels_pred*mask bf16 rows (b,t) padded 1600->1664,
                     13 rows of 80 per partition; 2080:2360 align half A fp8.
  dB u8 [128, 2488]: 0:2080 mels_target bf16 (same layout); 2080:2360 align
                     half B fp8; then the sidecar: stop/mask/iota f16 [13]
                     each in the split-per-b layout (b = p//64, t = 13*(p%64)+j),
                     length f32 of this partition's b, b-group indicator f32
                     [2], ones f32 (matmul rhs), stats prefill f32 [8]
                     (col 4 = lengths at partitions 0/1, rest 0).
Alignments: partition p = 16*b_local + i holds banded elements of b's 3 heads,
flattened, split 560 -> 280+280 between dA and dB.

Stop-term selection per b without transposes: per-partition masked-iota max
(mxp), a GpSimd partition all-reduce(max) over the per-b-masked [128,2]
candidates gives each partition its b's global max, then one is_equal*select.

Stats cols: 0=dc_w, 1=melA_lo, 2=sel_lnp, 3=mask_cnt, 4=len, 5=melA_hi, 6,7=0.
A PE matmul vs the ones column collapses [128,8] -> [8,1], DMA'd out.
"""

import numpy as np
import ml_dtypes

# Problem constants (hardcoded per contract; kernel.py must be self-contained).
H = 4
B = 16
T = 800
NMEL = 80
S = 160
N = 3
BW = 50
K = T // S  # 5
TC = 42  # band[:, t] == 0 for all t >= TC
NCORES = 8

MEL_ROWS = 2 * T            # 1600 (b,t) rows per core
MEL_PAD_ROWS = 1664         # pad to 128 * 13
MG = 13                     # 80-col groups per partition / stop t's per part
MEL_F = MG * NMEL           # 1040 mel elements per partition per tensor
MEL_LO = 520                # DVE reduces cols [0:520), ACT handles [520:1040)
ALN_PER_PLANE = 2975        # nonzero band cells per (n, bh) plane
ALN_PER_PART = 560          # 16*560 >= 3*2975, zero padded
ALN_HALF = ALN_PER_PART // 2  # 280

# dA byte offsets
AO_MP = 0
AO_ALN = 2 * MEL_F          # 2080
WA = AO_ALN + ALN_HALF      # 2360
# dB byte offsets
BO_MT = 0
BO_ALN = 2 * MEL_F          # 2080
BO_STOP = BO_ALN + ALN_HALF   # 2360
BO_MASK = BO_STOP + 2 * MG    # 2386
BO_IOTA = BO_MASK + 2 * MG    # 2412
BO_LEN = 2440                 # f32-aligned
BO_IND = 2444                 # 2 f32
BO_ONE = 2452                 # 1 f32
BO_ST = 2456                  # 8 f32 stats prefill
WB = BO_ST + 32               # 2488

_CACHE = {}


def _band_bool():
    tr = np.arange(TC)
    mn = np.clip(K * tr - BW, 0, S)
    mx = np.clip(K * tr + BW, 0, S)
    rows = np.arange(S)
    return (rows[:, None] >= mn[None, :]) & (rows[:, None] < mx[None, :])


def _build_bass():
    import concourse.bacc as bacc
    import concourse.tile as tile
    import concourse.mybir as mybir
    from concourse import bass_isa
    from contextlib import ExitStack

    f32 = mybir.dt.float32
    f16 = mybir.dt.float16
    bf16 = mybir.dt.bfloat16
    fp8 = mybir.dt.float8e4
    u8 = mybir.dt.uint8
    Alu = mybir.AluOpType
    Act = mybir.ActivationFunctionType
    Ax = mybir.AxisListType

    nc = bacc.Bacc("TRN2", target_bir_lowering=False, debug=False,
                   num_devices=NCORES)

    dA = nc.dram_tensor("dA", [128, WA], u8, kind="ExternalInput").ap()
    dB = nc.dram_tensor("dB", [128, WB], u8, kind="ExternalInput").ap()
    out = nc.dram_tensor("out", [8, 1], f32, kind="ExternalOutput").ap()

    with tile.TileContext(nc) as tc:
        with ExitStack() as ctx:
            pool = ctx.enter_context(tc.tile_pool(name="main", bufs=1))
            ppool = ctx.enter_context(tc.tile_pool(name="ps", bufs=1,
                                                   space="PSUM"))

            ta_t = pool.tile([128, WA], u8, tag="ta")
            tb_t = pool.tile([128, WB], u8, tag="tb")

            # DMA triggers first; SP and ACT drive separate HWDGE octets.
            nc.scalar.dma_start(tb_t[:], dB)
            nc.sync.dma_start(ta_t[:], dA)

            mp_v = ta_t[:, AO_MP:AO_ALN].bitcast(bf16)       # [128, 1040]
            alnA_v = ta_t[:, AO_ALN:WA].bitcast(fp8)         # [128, 280]
            mt_v = tb_t[:, BO_MT:BO_ALN].bitcast(bf16)
            alnB_v = tb_t[:, BO_ALN:BO_STOP].bitcast(fp8)
            stop_v = tb_t[:, BO_STOP:BO_MASK].bitcast(f16)   # [128, 13]
            mask_v = tb_t[:, BO_MASK:BO_IOTA].bitcast(f16)
            iota_v = tb_t[:, BO_IOTA:BO_IOTA + 2 * MG].bitcast(f16)
            lenf_v = tb_t[:, BO_LEN:BO_LEN + 4].bitcast(f32)
            ind_v = tb_t[:, BO_IND:BO_IND + 8].bitcast(f32)  # [128, 2]
            one_v = tb_t[:, BO_ONE:BO_ONE + 4].bitcast(f32)  # [128, 1]
            stats = tb_t[:, BO_ST:WB].bitcast(f32)           # [128, 8]

            # ---- stop term: split-per-b rows, GpSimd all-reduce argmax ----
            lnp_t = pool.tile([128, MG], f32, tag="lnp")
            nc.scalar.activation(lnp_t[:], stop_v, Act.Ln)
            tl_t = pool.tile([128, MG], f32, tag="tl")
            nc.vector.scalar_tensor_tensor(
                tl_t[:], iota_v, 1.0, mask_v, op0=Alu.bypass, op1=Alu.mult)
            mxp_t = pool.tile([128, 1], f32, tag="mxp")
            nc.vector.tensor_reduce(mxp_t[:], tl_t[:], axis=Ax.X, op=Alu.max)
            eq_t = pool.tile([128, MG], f32, tag="eq")
            cp_t = pool.tile([128, 1], f32, tag="cp")
            nc.vector.scalar_tensor_tensor(
                eq_t[:], tl_t[:], mxp_t[:, 0:1], lnp_t[:],
                op0=Alu.is_equal, op1=Alu.mult, accum_out=cp_t[:])
            mxi_t = pool.tile([128, 2], f32, tag="mxi")
            nc.vector.tensor_scalar(mxi_t[:], ind_v, mxp_t[:, 0:1], None,
                                    op0=Alu.mult)
            gmx2_t = pool.tile([128, 2], f32, tag="gmx2")
            nc.gpsimd.partition_all_reduce(gmx2_t[:], mxi_t[:], channels=128,
                                           reduce_op=bass_isa.ReduceOp.max)
            du2_t = pool.tile([128, 2], f32, tag="du2")
            gmx_t = pool.tile([128, 1], f32, tag="gmx")
            nc.vector.scalar_tensor_tensor(
                du2_t[:], gmx2_t[:], 1.0, ind_v,
                op0=Alu.bypass, op1=Alu.mult, accum_out=gmx_t[:])
            nc.vector.scalar_tensor_tensor(
                stats[:, 2:3], mxp_t[:], gmx_t[:, 0:1], cp_t[:],
                op0=Alu.is_equal, op1=Alu.mult)
            nc.vector.tensor_reduce(stats[:, 3:4], mask_v, axis=Ax.X,
                                    op=Alu.add)

            # ---- mel term: d = p*m - t; |.|-sum split across DVE and ACT ----
            d_t = pool.tile([128, MEL_F], bf16, tag="d")
            nc.vector.tensor_sub(d_t[:], mp_v, mt_v)
            nc.vector.tensor_reduce(stats[:, 1:2], d_t[:, 0:MEL_LO],
                                    axis=Ax.X, op=Alu.add,
                                    apply_absolute_value=True)
            da_t = pool.tile([128, MEL_F - MEL_LO], bf16, tag="da")
            nc.scalar.activation(da_t[:], d_t[:, MEL_LO:MEL_F], Act.Abs,
                                 accum_out=stats[:, 5:6])

            # ---- dc term: banded elements are pre-packed, just sum ----
            asum_t = pool.tile([128, ALN_HALF], bf16, tag="asum")
            dcs_t = pool.tile([128, 1], f32, tag="dcs")
            nc.vector.scalar_tensor_tensor(
                asum_t[:], alnA_v, 1.0, alnB_v,
                op0=Alu.bypass, op1=Alu.add, accum_out=dcs_t[:])
            bm_t = pool.tile([128, 1], f32, tag="bm")
            nc.vector.tensor_scalar(bm_t[:], lenf_v, float(T), None,
                                    op0=Alu.is_le)
            nc.vector.tensor_mul(stats[:, 0:1], dcs_t[:], bm_t[:])

            # ---- cross-partition reduce of stats via PE, then out ----
            pt = ppool.tile([8, 1], f32, tag="pt")
            nc.tensor.matmul(pt[:], lhsT=stats, rhs=one_v,
                             start=True, stop=True)
            ex_t = pool.tile([8, 1], f32, tag="ex")
            nc.vector.tensor_copy(ex_t[:], pt[:])
            nc.sync.dma_start(out, ex_t[:])

    nc.compile()
    return nc


def _get_nc():
    if "nc" not in _CACHE:
        _CACHE["nc"] = _build_bass()
    return _CACHE["nc"]


def make_in_maps(lengths, mask, stop_pred, mels_pred, mels_target, alignments):
    """Shard full inputs into the 8 per-core input dicts."""
    lengths = np.ascontiguousarray(lengths, dtype=np.int32)
    maskf = np.ascontiguousarray(mask).astype(np.float32)
    stop_pred = np.ascontiguousarray(stop_pred, dtype=np.float32)
    mels_pred = np.ascontiguousarray(mels_pred, dtype=np.float32)
    mels_target = np.ascontiguousarray(mels_target, dtype=np.float32)
    alignments = np.ascontiguousarray(alignments, dtype=np.float32)

    bf = ml_dtypes.bfloat16
    f8 = ml_dtypes.float8_e4m3
    band = _band_bool()  # [S, TC]
    el = alignments[:, :, :, :TC][:, :, band]  # [N, B*H, 2975]

    def split13(row, pad):
        o = np.full((64 * MG,), pad, row.dtype)
        o[:T] = row
        return o.reshape(64, MG)

    iota13 = np.concatenate([split13(np.arange(1, T + 1, dtype=np.float16),
                                     np.float16(0))] * 2)  # [128, 13]
    ind2 = np.zeros((128, 2), np.float32)
    ind2[:64, 0] = 1.0
    ind2[64:, 1] = 1.0

    def pad_rows(x2d):
        padded = np.zeros((MEL_PAD_ROWS, NMEL), x2d.dtype)
        padded[:MEL_ROWS] = x2d
        return padded.reshape(128, MEL_F)

    in_maps = []
    for c in range(NCORES):
        bs = slice(2 * c, 2 * c + 2)
        mp = (mels_pred[bs] * maskf[bs][..., None]).reshape(MEL_ROWS, NMEL)
        mt = mels_target[bs].reshape(MEL_ROWS, NMEL)

        aln = np.zeros((8, 16 * ALN_PER_PART), f8)
        core_el = el[:, 8 * c:8 * c + 8]          # [3, 8, 2975]
        aln[:, :N * ALN_PER_PLANE] = \
            core_el.transpose(1, 0, 2).reshape(8, N * ALN_PER_PLANE).astype(f8)
        aln = aln.reshape(128, ALN_PER_PART)

        dA = np.zeros((128, WA), np.uint8)
        dA[:, AO_MP:AO_ALN] = pad_rows(mp.astype(bf)).view(np.uint8)
        dA[:, AO_ALN:WA] = aln[:, :ALN_HALF].view(np.uint8)

        dB = np.zeros((128, WB), np.uint8)
        dB[:, BO_MT:BO_ALN] = pad_rows(mt.astype(bf)).view(np.uint8)
        dB[:, BO_ALN:BO_STOP] = np.ascontiguousarray(
            aln[:, ALN_HALF:]).view(np.uint8)
        st13 = np.concatenate(
            [split13(stop_pred[2 * c].astype(np.float16), np.float16(1.0)),
             split13(stop_pred[2 * c + 1].astype(np.float16), np.float16(1.0))])
        mk13 = np.concatenate(
            [split13(maskf[2 * c].astype(np.float16), np.float16(0)),
             split13(maskf[2 * c + 1].astype(np.float16), np.float16(0))])
        dB[:, BO_STOP:BO_MASK] = st13.view(np.uint8)
        dB[:, BO_MASK:BO_IOTA] = mk13.view(np.uint8)
        dB[:, BO_IOTA:BO_IOTA + 2 * MG] = iota13.view(np.uint8)
        lenf = np.repeat(lengths[bs].astype(np.float32), 64)  # [128]
        dB[:, BO_LEN:BO_LEN + 4] = lenf[:, None].view(np.uint8)
        dB[:, BO_IND:BO_IND + 8] = ind2.view(np.uint8)
        dB[:, BO_ONE:BO_ONE + 4] = np.ones((128, 1), np.float32).view(np.uint8)
        st = np.zeros((128, 8), np.float32)
        st[0:2, 4] = lengths[bs]
        dB[:, BO_ST:WB] = st.view(np.uint8)

        in_maps.append({"dA": dA, "dB": dB})
    return in_maps


def combine_partials(partials):
    """partials: list of 8 arrays [8,1] -> final scalar (0-d f32 ndarray)."""
    ps = np.stack([np.asarray(p, dtype=np.float64).reshape(8)
                   for p in partials])
    tot = ps.sum(axis=0)
    dc_w, sel_lnp, mask_cnt, len_sum = tot[0], tot[2], tot[3], tot[4]
    melA = tot[1] + tot[5]
    mel_loss = melA / float(B * T * NMEL)
    stop_loss = -5.0 * sel_lnp / mask_cnt
    dc = dc_w / (H * len_sum * N)
    return np.array(np.float32(mel_loss + stop_loss - 1e-4 * dc))


def kernel(lengths, mask, stop_pred, mels_pred, mels_target, alignments):
    from concourse.bass_utils import run_bass_kernel_spmd

    nc = _get_nc()
    in_maps = make_in_maps(lengths, np.asarray(mask), stop_pred,
                           mels_pred, mels_target, alignments)
    res = run_bass_kernel_spmd(nc, in_maps, list(range(NCORES)))
    return combine_partials([r["out"] for r in res.results])
